# revision 4
# baseline (speedup 1.0000x reference)
"""Trainium2 Bass kernel for Felsenstein pruning on a perfect binary tree
(nn_BaseLikelihoodModel: batched expm over edges + level-synchronous sweep).

Every edge matrix is expm(t_e * R) for ONE shared 16x16 rate matrix
R = Q - diag(growth).  A real block-eigendecomposition R = W M W^-1
(host-side, f64) turns the per-edge expm-matvec into

    expm(t R) v = W @ (EC(t) . u) + W2 @ (ES2(t) . u),   u = Winv @ v
    EC = e^{at} cos(bt),  ES2 = e^{at} sin(b2 t),  b2 = b[swap],  W2 = W @ Pswap

(the conjugate-pair permutation Pswap is folded into W2/b2 host-side, so
only ONE 128-wide matmul per level is needed for u).  The growth_rates
factor on right children is folded into the right-child weights gW/gW2.

The level sweep runs in probability space with one per-node rescale at
level 8 (accumulated log-scales), mathematically identical to the
reference's log-space logsumexp recursion.

Sharding: 8 contiguous subtrees of 4096 leaves (one per core); within a
core, 8 partition-blocks of 16 states hold 8 sub-subtrees of 512 leaves.
Children are ordered by bit-reversed local index, which keeps every
level's parents automatically left/right-separated (contiguous halves,
no strided PSUM reads).  Levels 10..12 pair partition blocks via
host-built block-diagonal selection weights.  Each core finishes by
applying its own root edge (level-13 child transform); the 15-node top
combine of the 8 per-core results happens on host during unsharding
(O(S^2) glue), so the kernel needs no collectives.

Branch lengths ship as one [8 x cols] row-block tensor and are broadcast
to all 128 partitions by a single 0/1 matmul; exp/cos/sin then run with
the per-partition a/b scales and BOOST/pi2 biases fused into the ACT op.
"""
import math
import numpy as np
import ml_dtypes

import concourse.bass as bass
import concourse.mybir as mybir
import concourse.tile as tile
from concourse.bass_utils import run_bass_kernel_spmd

F32 = np.float32
BF16 = ml_dtypes.bfloat16
S = 16
L = 32768
N = 2 * L
NCORES = 8
NBLK = 8
LPC = L // NCORES          # 4096 leaves per core
LPB = LPC // NBLK          # 512 leaves per block
BLK_W = [LPB >> hc for hc in range(9)]            # 512..2
BLK_OFF = np.concatenate([[0], np.cumsum(BLK_W)])  # offsets into 1022
BT_COLS = int(BLK_OFF[-1])                         # 1022
C_L10 = BT_COLS          # 1022..1024: block-domain steps, levels 10..12
C_L11 = BT_COLS + 1
C_L12 = BT_COLS + 2
C_OWN = BT_COLS + 3      # 1025: this core's root edge (level-13 child)
TC = C_OWN + 1           # 1026 total factor columns

OFFS = [0]
for _h in range(1, 16):
    OFFS.append(OFFS[-1] + (L >> (_h - 1)))

BOOST = 1.7
PI2 = float(np.pi / 2)
# device edges per core: 8 blocks x 1022 + (8+4+2) block steps + own root
DEV_EDGES = NBLK * BT_COLS + 14 + 1

# packA1 bf16 [128, 512]: [wTl | w2Tl | wTr | w2Tr]  (level Y weights)
A_WL, A_W2L, A_WR, A_W2R = 0, 128, 256, 384
# packB bf16 [128, 648]: [wLbd1T | wLbd2T | wRbd1T | wRbd2T | onesbd | onesc]
B_L1, B_L2, B_R1, B_R2, B_ONB, B_ONC = 0, 128, 256, 384, 512, 640
# packF f32 [128, 24]: cols avec,bvec,bvec2,ones8,BOOST,PI2,zero
F_A, F_B, F_B2, F_ONE8, F_BOOST, F_PI2, F_ZERO = 0, 1, 2, 3, 4, 5, 6


def _real_eig(R):
    """Real block eigendecomposition R = Wr @ M @ inv(Wr) with M block
    diagonal ([[a, b], [-b, a]] blocks for conjugate pairs)."""
    ev, V = np.linalg.eig(R)
    used = np.zeros(S, bool)
    order = np.argsort(-ev.real)
    cols = []
    for i in order:
        if used[i]:
            continue
        if abs(ev[i].imag) < 1e-12:
            cols.append(("real", i))
            used[i] = True
        else:
            j = None
            for i2 in order:
                if not used[i2] and i2 != i and abs(ev[i2] - ev[i].conj()) < 1e-8:
                    j = i2
                    break
            assert j is not None, "unpaired complex eigenvalue"
            ip = i if ev[i].imag > 0 else j
            cols.append(("pair", ip))
            used[i] = used[j] = True
    Wr = np.zeros((S, S))
    a = np.zeros(S)
    bsig = np.zeros(S)
    swap = np.arange(S)
    k = 0
    for c in cols:
        if c[0] == "real":
            i = c[1]
            Wr[:, k] = V[:, i].real
            a[k] = ev[i].real
            k += 1
        else:
            ip = c[1]
            lam = ev[ip]
            Wr[:, k] = V[:, ip].real
            Wr[:, k + 1] = V[:, ip].imag
            a[k] = a[k + 1] = lam.real
            bsig[k] = lam.imag
            bsig[k + 1] = -lam.imag
            swap[k] = k + 1
            swap[k + 1] = k
            k += 2
    assert k == S
    scales = np.ones(S)
    kk = 0
    while kk < S:
        if swap[kk] == kk:
            scales[kk] = np.linalg.norm(Wr[:, kk])
            kk += 1
        else:
            s = math.sqrt(np.linalg.norm(Wr[:, kk]) * np.linalg.norm(Wr[:, kk + 1]))
            scales[kk] = scales[kk + 1] = s
            kk += 2
    Wr = Wr / scales[None, :]
    Winv = np.linalg.inv(Wr)
    return Wr, Winv, a, bsig, swap


def _split_multi_waits(nc):
    """Walrus codegen allows only ONE sync-wait slot per engine instruction;
    move extras onto prepended same-engine NoOps."""
    skip = (mybir.InstAllEngineBarrier, mybir.InstBranchHint,
            mybir.InstCompareAndBranch, mybir.InstUnconditionalBranch,
            mybir.InstIndirectBranch)
    for fn in nc.m.functions:
        for blk in fn.blocks:
            out = []
            for inst in blk.instructions:
                si = inst.sync_info
                if (si is not None and si.on_wait and len(si.on_wait) > 1
                        and not isinstance(inst, skip)):
                    waits = list(si.on_wait)
                    for i, w in enumerate(waits[:-1]):
                        nop = mybir.InstNoOp(
                            name=f"{inst.name}-wait{i}", ins=[], outs=[])
                        nop.engine = inst.engine
                        nop.sync_info = mybir.SyncInfo(
                            on_wait=[w], on_update=[])
                        out.append(nop)
                    inst.sync_info = mybir.SyncInfo(
                        on_wait=[waits[-1]], on_update=list(si.on_update or []))
                out.append(inst)
            blk.instructions = out


def _bitrev(n):
    """bit-reversal permutation of 0..n-1.  Ordering children by bit-
    reversed local index makes every level's parents emerge already
    left/right-separated for the next level (o(k)/2 = bitrev_{m-1}(k)),
    so the sweep only ever reads contiguous column halves."""
    m = int(np.log2(n))
    idx = np.arange(n)
    out = np.zeros(n, dtype=np.int64)
    for b in range(m):
        out |= ((idx >> b) & 1) << (m - 1 - b)
    return out


def _host_prep(branch_lens, init_partials, Q, growth_rates):
    bl = np.ascontiguousarray(np.asarray(branch_lens, dtype=F32))
    ip = np.ascontiguousarray(np.asarray(init_partials, dtype=F32))
    Q64 = np.asarray(Q, dtype=np.float64)
    g64 = np.asarray(growth_rates, dtype=np.float64)
    R = Q64 - np.diag(g64)
    Wr, Winv, a, bsig, swap = _real_eig(R)
    P = np.zeros((S, S))
    P[np.arange(S), swap] = 1.0
    W2 = Wr @ P
    b2 = bsig[swap]
    gW = np.diag(g64) @ Wr
    gW2 = np.diag(g64) @ W2
    I8 = np.eye(8)

    def bf(x):
        return np.ascontiguousarray(
            np.asarray(x, dtype=np.float32).astype(BF16))

    # b2 = bsig[swap] is exactly -bsig, so ES2 = -ES: drop the second
    # sin pass on device and negate every W2-side weight instead
    packA1 = bf(np.concatenate([
        np.kron(I8, Wr.T), np.kron(I8, -W2.T),
        np.kron(I8, gW.T), np.kron(I8, -gW2.T)], axis=1))
    packA2 = bf(np.kron(I8, Winv.T))

    def bd(M, right):
        out = np.zeros((128, 128))
        for p in range(4):
            c0 = 32 * p + (16 if right else 0)
            out[16 * p:16 * p + 16, c0:c0 + 16] = M
        return out.T

    packB = bf(np.concatenate([
        bd(Wr, False), bd(-W2, False), bd(gW, True), bd(-gW2, True),
        np.kron(I8, np.ones((S, S))), np.kron(I8, np.ones((S, 1)))], axis=1))

    packF = np.zeros((128, 24), dtype=F32)
    packF[:, F_A] = np.tile(a, 8)
    packF[:, F_B] = np.tile(bsig, 8)
    packF[:, F_B2] = np.tile(b2, 8)
    packF[0:8, F_ONE8] = 1.0
    packF[:, F_BOOST] = BOOST
    packF[:, F_PI2] = PI2
    packF = np.ascontiguousarray(packF)

    expd = bf(np.kron(I8, np.ones((1, S))))   # [8, 128] 0/1 broadcast

    states = np.argmax(ip[:L], axis=1)  # leaf one-hots: 0.0 at state, NEG
    Winvb = np.asarray(Winv, dtype=F32).astype(BF16)
    di512 = _bitrev(512)

    in_maps = []
    for c in range(NCORES):
        st = states[c * LPC:(c + 1) * LPC].reshape(8, 512)[:, di512]
        ul = Winvb[:, st]                       # [16, 8, 512]
        u_leaf = np.ascontiguousarray(ul.transpose(1, 0, 2).reshape(128, 512))

        T8 = np.zeros((8, TC), dtype=F32)
        for hc in range(9):          # levels 1..9, children = level-hc nodes
            w = LPB >> hc
            base = OFFS[hc] + c * (LPC >> hc)
            seg = bl[base: base + (LPC >> hc)].reshape(8, w)
            T8[:, int(BLK_OFF[hc]): int(BLK_OFF[hc]) + w] = seg[:, _bitrev(w)]
        T8[:, C_L10] = bl[OFFS[9] + c * 8: OFFS[9] + c * 8 + 8]
        T8[0:4, C_L11] = bl[OFFS[10] + c * 4: OFFS[10] + c * 4 + 4]
        T8[0:2, C_L12] = bl[OFFS[11] + c * 2: OFFS[11] + c * 2 + 2]
        T8[0, C_OWN] = bl[OFFS[12] + c]

        # per-core level-13 child weights: left (even core) vs right (odd)
        if c % 2 == 0:
            wt = np.concatenate([Wr.T, -W2.T], axis=1)
        else:
            wt = np.concatenate([gW.T, -gW2.T], axis=1)

        t8e = np.concatenate([T8, np.zeros((8, 128), F32)], axis=1)
        t8e = bf(t8e)
        t8e[:, TC:TC + 128] = expd
        in_maps.append({"t8e": t8e, "u_leaf": u_leaf, "wtail": bf(wt),
                        "packA1": packA1, "packA2": packA2,
                        "packB": packB, "packF": packF})
    return in_maps


def build_nc(split_waits=True):
    f32 = mybir.dt.float32
    bf16 = mybir.dt.bfloat16
    AF = mybir.ActivationFunctionType
    OP = mybir.AluOpType
    nc = bass.Bass()

    t8e = nc.dram_tensor("t8e", [8, TC + 128], bf16, kind="ExternalInput")
    u_leaf = nc.dram_tensor("u_leaf", [128, 512], bf16, kind="ExternalInput")
    wtail = nc.dram_tensor("wtail", [16, 32], bf16, kind="ExternalInput")
    packA1 = nc.dram_tensor("packA1", [128, 512], bf16, kind="ExternalInput")
    packA2 = nc.dram_tensor("packA2", [128, 128], bf16, kind="ExternalInput")
    packB = nc.dram_tensor("packB", [128, 648], bf16, kind="ExternalInput")
    packF = nc.dram_tensor("packF", [128, 24], f32, kind="ExternalInput")
    out = nc.dram_tensor("out", [17, 1], f32, kind="ExternalOutput")

    with tile.TileContext(nc) as tc:
        with (
            tc.tile_pool(name="const", bufs=1) as cp,
            tc.tile_pool(name="sb", bufs=2) as sb,
            tc.tile_pool(name="big", bufs=1) as bigp,
            tc.tile_pool(name="psT", bufs=1, space="PSUM") as psT,
            tc.tile_pool(name="psU", bufs=1, space="PSUM") as psU,
            tc.tile_pool(name="psL", bufs=1, space="PSUM") as psL,
            tc.tile_pool(name="psR", bufs=1, space="PSUM") as psR,
            tc.tile_pool(name="psS", bufs=1, space="PSUM") as psS,
        ):
            # --- table preload: tiny Exp fires the exp-table DMA at t=0
            d0 = cp.tile([1, 1], f32, tag="d0")
            nc.vector.memset(d0[:], 0.0)
            dm = cp.tile([1, 1], f32, tag="dm")
            nc.scalar.activation(dm[:], d0[:], AF.Exp, bias=d0[0:1, 0:1])

            # --- early inputs (tiny): branch rows, broadcast matrix, biases
            sTE = cp.tile([8, TC + 128], bf16, tag="t8e")
            nc.sync.dma_start(sTE[:], t8e[:, :])
            sT8 = sTE[:, 0:TC]
            cE = sTE[:, TC:TC + 128]
            cF = cp.tile([128, 24], f32, tag="packF")
            nc.sync.dma_start(cF[:], packF[:, :])

            # --- broadcast t to all 128 partitions (one matmul per chunk,
            # bank-aligned PSUM tiles); chunk order = consumption order
            # (level-1 right half first)
            T0 = psT.tile([128, 512], f32, tag="T0")
            T1 = psT.tile([128, 512], f32, tag="T1")
            T2 = psT.tile([128, TC - 1024], f32, tag="T2")
            TCH = ((256, 256, T0, 256), (0, 256, T0, 0),
                   (512, 512, T1, 0), (1024, TC - 1024, T2, 0))
            for lo, w, tt, to in TCH:
                nc.tensor.matmul(tt[:, to:to + w], cE, sT8[:, lo:lo + w],
                                 start=True, stop=True)

            # --- edge factors: exp all (table resident via the dummy),
            # then sin all (one load), scales/biases fused into ACT
            sE = bigp.tile([128, TC], f32, tag="sE")
            sC = bigp.tile([128, TC], f32, tag="sC")
            sSn = bigp.tile([128, TC], f32, tag="sSn")
            for lo, w, tt, to in TCH:
                nc.scalar.activation(sE[:, lo:lo + w], tt[:, to:to + w],
                                     AF.Exp, bias=cF[:, F_BOOST:F_BOOST + 1],
                                     scale=cF[:, F_A:F_A + 1])
            for lo, w, tt, to in TCH:
                nc.scalar.activation(sC[:, lo:lo + w], tt[:, to:to + w],
                                     AF.Sin, bias=cF[:, F_PI2:F_PI2 + 1],
                                     scale=cF[:, F_B:F_B + 1])
                nc.scalar.activation(sSn[:, lo:lo + w], tt[:, to:to + w],
                                     AF.Sin, bias=cF[:, F_ZERO:F_ZERO + 1],
                                     scale=cF[:, F_B:F_B + 1])

            # --- big inputs stream in behind the tiny ones
            cU = cp.tile([128, 512], bf16, tag="u_leaf")
            nc.sync.dma_start(cU[:], u_leaf[:, :])
            cA1 = cp.tile([128, 512], bf16, tag="packA1")
            nc.sync.dma_start(cA1[:], packA1[:, :])
            cA2 = cp.tile([128, 128], bf16, tag="packA2")
            nc.sync.dma_start(cA2[:], packA2[:, :])
            cB = cp.tile([128, 648], bf16, tag="packB")
            nc.sync.dma_start(cB[:], packB[:, :])
            cWt = cp.tile([16, 32], bf16, tag="wtail")
            nc.sync.dma_start(cWt[:], wtail[:, :])
            pobs1 = psL.tile([1, 1], f32, tag="YL")
            nc.tensor.matmul(pobs1[:], cU[0:1, 0:1], cU[0:1, 0:1],
                             start=True, stop=False)
            nc.tensor.matmul(pobs1[:], cA1[0:1, 0:1], cA1[0:1, 0:1],
                             start=False, stop=False)
            nc.tensor.matmul(pobs1[:], cA2[0:1, 0:1], cA2[0:1, 0:1],
                             start=False, stop=True)
            pobs2 = psR.tile([1, 1], f32, tag="YR")
            nc.tensor.matmul(pobs2[:], cB[0:1, 0:1], cB[0:1, 0:1],
                             start=True, stop=False)
            nc.tensor.matmul(pobs2[:], cF[0:1, 0:1], cF[0:1, 0:1],
                             start=False, stop=True)

            EC = bigp.tile([128, TC], f32, tag="EC")
            ES = bigp.tile([128, TC], f32, tag="ES")

            def eces(lo, w):
                nc.vector.tensor_mul(EC[:, lo:lo + w], sE[:, lo:lo + w],
                                     sC[:, lo:lo + w])
                nc.vector.tensor_mul(ES[:, lo:lo + w], sE[:, lo:lo + w],
                                     sSn[:, lo:lo + w])

            wl = cA1[:, A_WL:A_WL + 128]
            w2l = cA1[:, A_W2L:A_W2L + 128]
            wr = cA1[:, A_WR:A_WR + 128]
            w2r = cA1[:, A_W2R:A_W2R + 128]

            def level_cols(V, lo, wp, uL=None, uR=None, out_dt=bf16):
                """One sweep level, 128 partitions, contiguous halves.
                The u-matmul runs right-half first so the m-chain starts
                after a half-width matmul; the YR bounce is a DVE copy
                (cheaper than ACT and one fewer cross-engine hop)."""
                if V is not None:
                    pU = psU.tile([128, 2 * wp], f32, tag="U")
                    nc.tensor.matmul(pU[:, wp:2 * wp], cA2[:],
                                     V[:, wp:2 * wp], start=True, stop=True)
                    nc.tensor.matmul(pU[:, 0:wp], cA2[:], V[:, 0:wp],
                                     start=True, stop=True)
                    uL, uR = pU[:, 0:wp], pU[:, wp:2 * wp]
                m1R = sb.tile([128, wp], bf16, tag="m1R")
                nc.vector.tensor_mul(m1R[:], EC[:, lo + wp:lo + 2 * wp], uR)
                m2R = sb.tile([128, wp], bf16, tag="m2R")
                nc.vector.tensor_mul(m2R[:], ES[:, lo + wp:lo + 2 * wp], uR)
                m1L = sb.tile([128, wp], bf16, tag="m1L")
                nc.vector.tensor_mul(m1L[:], EC[:, lo:lo + wp], uL)
                m2L = sb.tile([128, wp], bf16, tag="m2L")
                nc.vector.tensor_mul(m2L[:], ES[:, lo:lo + wp], uL)
                pYR = psR.tile([128, wp], f32, tag="YR")
                nc.tensor.matmul(pYR[:], wr, m1R[:], start=True, stop=False)
                nc.tensor.matmul(pYR[:], w2r, m2R[:], start=False, stop=True)
                sYR = sb.tile([128, wp], bf16, tag="sYR")
                nc.scalar.activation(sYR[:], pYR[:], AF.Copy)
                pYL = psL.tile([128, wp], f32, tag="YL")
                nc.tensor.matmul(pYL[:], wl, m1L[:], start=True, stop=False)
                nc.tensor.matmul(pYL[:], w2l, m2L[:], start=False, stop=True)
                Vn = sb.tile([128, wp], out_dt, tag="V")
                nc.vector.tensor_mul(Vn[:], pYL[:], sYR[:])
                return Vn

            # --- level 1: leaf u-vectors come from the host; EC/ES chunks
            # interleave with the m-multiplies in consumption order, all
            # on DVE (concurrent GPSIMD traffic slows both ~2.5x)
            eces(256, 256)
            m1R = sb.tile([128, 256], bf16, tag="m1R")
            nc.vector.tensor_mul(m1R[:], EC[:, 256:512], cU[:, 256:512])
            m2R = sb.tile([128, 256], bf16, tag="m2R")
            nc.vector.tensor_mul(m2R[:], ES[:, 256:512], cU[:, 256:512])
            eces(0, 256)
            m1L = sb.tile([128, 256], bf16, tag="m1L")
            nc.vector.tensor_mul(m1L[:], EC[:, 0:256], cU[:, 0:256])
            m2L = sb.tile([128, 256], bf16, tag="m2L")
            nc.vector.tensor_mul(m2L[:], ES[:, 0:256], cU[:, 0:256])
            pYR = psR.tile([128, 256], f32, tag="YR")
            nc.tensor.matmul(pYR[:], wr, m1R[:], start=True, stop=False)
            nc.tensor.matmul(pYR[:], w2r, m2R[:], start=False, stop=True)
            sYR = sb.tile([128, 256], bf16, tag="sYR")
            nc.scalar.activation(sYR[:], pYR[:], AF.Copy)
            pYL = psL.tile([128, 256], f32, tag="YL")
            nc.tensor.matmul(pYL[:], wl, m1L[:], start=True, stop=False)
            nc.tensor.matmul(pYL[:], w2l, m2L[:], start=False, stop=True)
            V = sb.tile([128, 256], bf16, tag="V")
            nc.vector.tensor_mul(V[:], pYL[:], sYR[:])

            # --- levels 2..9; remaining EC/ES chunks slot in just ahead
            # of the level that first needs them
            lsW = None
            for h in range(2, 10):
                wc = BLK_W[h - 1]
                wp = wc // 2
                lo = int(BLK_OFF[h - 1])
                if h == 2:
                    eces(512, 256)
                elif h == 3:
                    eces(768, TC - 768)
                if h == 8:
                    praw = level_cols(V, lo, wp)
                    pSb = psU.tile([128, wp], f32, tag="U")
                    nc.tensor.matmul(pSb[:], cB[:, B_ONB:B_ONB + 128],
                                     praw[:], start=True, stop=True)
                    pSc = psS.tile([8, wp], f32, tag="S")
                    nc.tensor.matmul(pSc[:], cB[:, B_ONC:B_ONC + 8],
                                     praw[:], start=True, stop=True)
                    rb = sb.tile([128, wp], f32, tag="rb")
                    nc.vector.reciprocal(rb[:], pSb[:])
                    V = sb.tile([128, wp], bf16, tag="V")
                    nc.vector.tensor_mul(V[:], praw[:], rb[:])
                    lsW = sb.tile([8, wp], f32, tag="lsW")
                    nc.scalar.activation(lsW[:], pSc[:], AF.Ln)
                else:
                    V = level_cols(V, lo, wp)

            # logscale: fold h=8's two parents, then sum the 8 blocks
            ls9 = sb.tile([8, 1], f32, tag="ls9")
            nc.gpsimd.tensor_add(ls9[:], lsW[:, 0:1], lsW[:, 1:2])
            ptot = psS.tile([1, 1], f32, tag="S")
            nc.tensor.matmul(ptot[:], cF[0:8, F_ONE8:F_ONE8 + 1], ls9[:],
                             start=True, stop=True)
            stot = sb.tile([1, 1], f32, tag="stot")
            nc.vector.tensor_copy(stot[:], ptot[:])
            nc.sync.dma_start(out[16:17, 0:1], stot[:])

            # --- levels 10..12 (block domain: pairs of partition blocks)
            col = C_L10
            for kp, op in ((128, 64), (64, 32), (32, 16)):
                pU = psU.tile([kp, 1], f32, tag="U")
                nc.tensor.matmul(pU[:], cA2[0:kp, 0:kp], V[:],
                                 start=True, stop=True)
                m1 = sb.tile([kp, 1], bf16, tag="m1R")
                nc.vector.tensor_mul(m1[:], EC[0:kp, col:col + 1], pU[:])
                m2 = sb.tile([kp, 1], bf16, tag="m2R")
                nc.vector.tensor_mul(m2[:], ES[0:kp, col:col + 1], pU[:])
                pYR = psR.tile([op, 1], f32, tag="YR")
                nc.tensor.matmul(pYR[:], cB[0:kp, B_R1:B_R1 + op], m1[:],
                                 start=True, stop=False)
                nc.tensor.matmul(pYR[:], cB[0:kp, B_R2:B_R2 + op], m2[:],
                                 start=False, stop=True)
                sYR = sb.tile([op, 1], bf16, tag="sYR")
                nc.vector.tensor_copy(sYR[:], pYR[:])
                pYL = psL.tile([op, 1], f32, tag="YL")
                nc.tensor.matmul(pYL[:], cB[0:kp, B_L1:B_L1 + op], m1[:],
                                 start=True, stop=False)
                nc.tensor.matmul(pYL[:], cB[0:kp, B_L2:B_L2 + op], m2[:],
                                 start=False, stop=True)
                V = sb.tile([op, 1], bf16, tag="V")
                nc.vector.tensor_mul(V[:], pYL[:], sYR[:])
                col += 1

            # --- own level-13 child transform (this core's root edge;
            # W vs gW by core parity baked into the wtail input)
            pU = psU.tile([16, 1], f32, tag="U")
            nc.tensor.matmul(pU[:], cA2[0:16, 0:16], V[:],
                             start=True, stop=True)
            m1 = sb.tile([16, 1], bf16, tag="m1R")
            nc.vector.tensor_mul(m1[:], EC[0:16, C_OWN:C_OWN + 1], pU[:])
            m2 = sb.tile([16, 1], bf16, tag="m2R")
            nc.vector.tensor_mul(m2[:], ES[0:16, C_OWN:C_OWN + 1], pU[:])
            pYo = psL.tile([16, 1], f32, tag="YL")
            nc.tensor.matmul(pYo[:], cWt[:, 0:16], m1[:],
                             start=True, stop=False)
            nc.tensor.matmul(pYo[:], cWt[:, 16:32], m2[:],
                             start=False, stop=True)
            yown = sb.tile([16, 1], f32, tag="yown")
            nc.vector.tensor_copy(yown[:], pYo[:])
            nc.sync.dma_start(out[0:16, 0:1], yown[:])

    if split_waits:
        _split_multi_waits(nc)
    return nc


def _host_top_combine(results, branch_lens, Q, growth_rates):
    """Final 7-node combine (levels 14,15 + root) of the per-core
    contributions, in f64 -- O(S^2) unsharding glue."""
    bl = np.asarray(branch_lens, dtype=np.float64)
    Q64 = np.asarray(Q, dtype=np.float64)
    g64 = np.asarray(growth_rates, dtype=np.float64)
    R = Q64 - np.diag(g64)
    Wr, Winv, a, bsig, swap = _real_eig(R)
    P = np.zeros((S, S))
    P[np.arange(S), swap] = 1.0
    W2 = Wr @ P
    b2 = bsig[swap]

    def edge(t, v, g=None):
        u = Winv @ v
        y = Wr @ (np.exp(a * t) * np.cos(bsig * t) * u) \
            + W2 @ (np.exp(a * t) * np.sin(b2 * t) * u)
        return y if g is None else g * y

    ys = [np.asarray(r["out"], dtype=np.float64)[0:16, 0] for r in results]
    tot = sum(float(np.asarray(r["out"], dtype=np.float64)[16, 0])
              for r in results)
    corr = np.float64(NCORES * DEV_EDGES) * np.float64(np.float32(BOOST))
    v14 = [ys[2 * j] * ys[2 * j + 1] for j in range(4)]
    y14 = [edge(bl[OFFS[13] + k], v14[k], g64 if k % 2 else None)
           for k in range(4)]
    v15 = [y14[0] * y14[1], y14[2] * y14[3]]
    y15 = [edge(bl[OFFS[14] + k], v15[k], g64 if k % 2 else None)
           for k in range(2)]
    v16 = y15[0] * y15[1]
    yroot = edge(bl[OFFS[15]], v16)
    return (np.log(yroot) + (tot - corr)).astype(F32)


def kernel(postorder, children, parents, branch_lens, init_partials, Q,
           levels, growth_rates, *, _trace=False):
    in_maps = _host_prep(branch_lens, init_partials, Q, growth_rates)
    nc = build_nc()
    res = run_bass_kernel_spmd(nc, in_maps, core_ids=list(range(NCORES)),
                               trace=_trace)
    out = _host_top_combine(res.results, branch_lens, Q, growth_rates)
    if _trace:
        kernel.last_exec_time_ns = res.exec_time_ns
        kernel.last_results = res
    return out


# revision 5
# speedup vs baseline: 1.2052x; 1.2052x over previous
"""Trainium2 Bass kernel for Felsenstein pruning on a perfect binary tree
(nn_BaseLikelihoodModel: batched expm over edges + level-synchronous sweep).

Every edge matrix is expm(t_e * R) for ONE shared 16x16 rate matrix
R = Q - diag(growth).  A real block-eigendecomposition R = W M W^-1
(host-side, f64) turns the per-edge expm-matvec into

    expm(t R) v = W @ (EC(t) . u) + W2 @ (ES2(t) . u),   u = Winv @ v
    EC = e^{at} cos(bt),  ES2 = e^{at} sin(b2 t),  b2 = b[swap],  W2 = W @ Pswap

(the conjugate-pair permutation Pswap is folded into W2/b2 host-side, so
only ONE 128-wide matmul per level is needed for u).  The growth_rates
factor on right children is folded into the right-child weights gW/gW2.

The level sweep runs in probability space with one per-node rescale at
level 8 (accumulated log-scales), mathematically identical to the
reference's log-space logsumexp recursion.

Sharding: 8 contiguous subtrees of 4096 leaves (one per core); within a
core, 8 partition-blocks of 16 states hold 8 sub-subtrees of 512 leaves.
Children are ordered by bit-reversed local index, which keeps every
level's parents automatically left/right-separated (contiguous halves,
no strided PSUM reads).  Levels 10..12 pair partition blocks via
host-built block-diagonal selection weights.  Each core finishes by
applying its own root edge (level-13 child transform); the 15-node top
combine of the 8 per-core results happens on host during unsharding
(O(S^2) glue), so the kernel needs no collectives.

Branch lengths ship as one [8 x cols] row-block tensor and are broadcast
to all 128 partitions by a single 0/1 matmul; exp/cos/sin then run with
the per-partition a/b scales and BOOST/pi2 biases fused into the ACT op.
"""
import math
import numpy as np
import ml_dtypes

import concourse.bass as bass
import concourse.mybir as mybir
import concourse.tile as tile
from concourse.bass_utils import run_bass_kernel_spmd

F32 = np.float32
BF16 = ml_dtypes.bfloat16
S = 16
L = 32768
N = 2 * L
NCORES = 8
NBLK = 8
LPC = L // NCORES          # 4096 leaves per core
LPB = LPC // NBLK          # 512 leaves per block
BLK_W = [LPB >> hc for hc in range(9)]            # 512..2
BLK_OFF = np.concatenate([[0], np.cumsum(BLK_W)])  # offsets into 1022
BT_COLS = int(BLK_OFF[-1])                         # 1022
C_L10 = BT_COLS          # 1022..1024: block-domain steps, levels 10..12
C_L11 = BT_COLS + 1
C_L12 = BT_COLS + 2
C_OWN = BT_COLS + 3      # 1025: this core's root edge (level-13 child)
TC = C_OWN + 1           # 1026 total factor columns

OFFS = [0]
for _h in range(1, 16):
    OFFS.append(OFFS[-1] + (L >> (_h - 1)))

BOOST = 1.7
PI2 = float(np.pi / 2)
# device edges per core: 8 blocks x 1022 + (8+4+2) block steps + own root
DEV_EDGES = NBLK * BT_COLS + 14 + 1

# packA1 bf16 [128, 512]: [wTl | w2Tl | wTr | w2Tr]  (level Y weights)
A_WL, A_W2L, A_WR, A_W2R = 0, 128, 256, 384
# packB bf16 [128, 648]: [wLbd1T | wLbd2T | wRbd1T | wRbd2T | onesbd | onesc]
B_L1, B_L2, B_R1, B_R2, B_ONB, B_ONC = 0, 128, 256, 384, 512, 640
# packF f32 [128, 24]: cols avec,bvec,bvec2,ones8,BOOST,PI2,zero
F_A, F_B, F_B2, F_ONE8, F_BOOST, F_PI2, F_ZERO = 0, 1, 2, 3, 4, 5, 6


def _real_eig(R):
    """Real block eigendecomposition R = Wr @ M @ inv(Wr) with M block
    diagonal ([[a, b], [-b, a]] blocks for conjugate pairs)."""
    ev, V = np.linalg.eig(R)
    used = np.zeros(S, bool)
    order = np.argsort(-ev.real)
    cols = []
    for i in order:
        if used[i]:
            continue
        if abs(ev[i].imag) < 1e-12:
            cols.append(("real", i))
            used[i] = True
        else:
            j = None
            for i2 in order:
                if not used[i2] and i2 != i and abs(ev[i2] - ev[i].conj()) < 1e-8:
                    j = i2
                    break
            assert j is not None, "unpaired complex eigenvalue"
            ip = i if ev[i].imag > 0 else j
            cols.append(("pair", ip))
            used[i] = used[j] = True
    Wr = np.zeros((S, S))
    a = np.zeros(S)
    bsig = np.zeros(S)
    swap = np.arange(S)
    k = 0
    for c in cols:
        if c[0] == "real":
            i = c[1]
            Wr[:, k] = V[:, i].real
            a[k] = ev[i].real
            k += 1
        else:
            ip = c[1]
            lam = ev[ip]
            Wr[:, k] = V[:, ip].real
            Wr[:, k + 1] = V[:, ip].imag
            a[k] = a[k + 1] = lam.real
            bsig[k] = lam.imag
            bsig[k + 1] = -lam.imag
            swap[k] = k + 1
            swap[k + 1] = k
            k += 2
    assert k == S
    scales = np.ones(S)
    kk = 0
    while kk < S:
        if swap[kk] == kk:
            scales[kk] = np.linalg.norm(Wr[:, kk])
            kk += 1
        else:
            s = math.sqrt(np.linalg.norm(Wr[:, kk]) * np.linalg.norm(Wr[:, kk + 1]))
            scales[kk] = scales[kk + 1] = s
            kk += 2
    Wr = Wr / scales[None, :]
    Winv = np.linalg.inv(Wr)
    return Wr, Winv, a, bsig, swap


def _split_multi_waits(nc):
    """Walrus codegen allows only ONE sync-wait slot per engine instruction;
    move extras onto prepended same-engine NoOps."""
    skip = (mybir.InstAllEngineBarrier, mybir.InstBranchHint,
            mybir.InstCompareAndBranch, mybir.InstUnconditionalBranch,
            mybir.InstIndirectBranch)
    for fn in nc.m.functions:
        for blk in fn.blocks:
            out = []
            for inst in blk.instructions:
                si = inst.sync_info
                if (si is not None and si.on_wait and len(si.on_wait) > 1
                        and not isinstance(inst, skip)):
                    waits = list(si.on_wait)
                    for i, w in enumerate(waits[:-1]):
                        nop = mybir.InstNoOp(
                            name=f"{inst.name}-wait{i}", ins=[], outs=[])
                        nop.engine = inst.engine
                        nop.sync_info = mybir.SyncInfo(
                            on_wait=[w], on_update=[])
                        out.append(nop)
                    inst.sync_info = mybir.SyncInfo(
                        on_wait=[waits[-1]], on_update=list(si.on_update or []))
                out.append(inst)
            blk.instructions = out


def _bitrev(n):
    """bit-reversal permutation of 0..n-1.  Ordering children by bit-
    reversed local index makes every level's parents emerge already
    left/right-separated for the next level (o(k)/2 = bitrev_{m-1}(k)),
    so the sweep only ever reads contiguous column halves."""
    m = int(np.log2(n))
    idx = np.arange(n)
    out = np.zeros(n, dtype=np.int64)
    for b in range(m):
        out |= ((idx >> b) & 1) << (m - 1 - b)
    return out


def _host_prep(branch_lens, init_partials, Q, growth_rates):
    bl = np.ascontiguousarray(np.asarray(branch_lens, dtype=F32))
    ip = np.ascontiguousarray(np.asarray(init_partials, dtype=F32))
    Q64 = np.asarray(Q, dtype=np.float64)
    g64 = np.asarray(growth_rates, dtype=np.float64)
    R = Q64 - np.diag(g64)
    Wr, Winv, a, bsig, swap = _real_eig(R)
    P = np.zeros((S, S))
    P[np.arange(S), swap] = 1.0
    W2 = Wr @ P
    b2 = bsig[swap]
    gW = np.diag(g64) @ Wr
    gW2 = np.diag(g64) @ W2
    I8 = np.eye(8)

    def bf(x):
        return np.ascontiguousarray(
            np.asarray(x, dtype=np.float32).astype(BF16))

    # b2 = bsig[swap] is exactly -bsig, so ES2 = -ES: drop the second
    # sin pass on device and negate every W2-side weight instead
    packA1 = bf(np.concatenate([
        np.kron(I8, Wr.T), np.kron(I8, -W2.T),
        np.kron(I8, gW.T), np.kron(I8, -gW2.T)], axis=1))
    packA2 = bf(np.kron(I8, Winv.T))

    def bd(M, right):
        out = np.zeros((128, 128))
        for p in range(4):
            c0 = 32 * p + (16 if right else 0)
            out[16 * p:16 * p + 16, c0:c0 + 16] = M
        return out.T

    packB = bf(np.concatenate([
        bd(Wr, False), bd(-W2, False), bd(gW, True), bd(-gW2, True),
        np.kron(I8, np.ones((S, S))), np.kron(I8, np.ones((S, 1)))], axis=1))

    packF = np.zeros((128, 24), dtype=F32)
    packF[:, F_A] = np.tile(a, 8)
    packF[:, F_B] = np.tile(bsig, 8)
    packF[:, F_B2] = np.tile(b2, 8)
    packF[0:8, F_ONE8] = 1.0
    packF[:, F_BOOST] = BOOST
    packF[:, F_PI2] = PI2
    packF = np.ascontiguousarray(packF)

    expd = bf(np.kron(I8, np.ones((1, S))))   # [8, 128] 0/1 broadcast

    states = np.argmax(ip[:L], axis=1)  # leaf one-hots: 0.0 at state, NEG
    Winvb = np.asarray(Winv, dtype=F32).astype(BF16)
    di512 = _bitrev(512)

    in_maps = []
    for c in range(NCORES):
        st = states[c * LPC:(c + 1) * LPC].reshape(8, 512)[:, di512]
        ul = Winvb[:, st]                       # [16, 8, 512]
        u_leaf = np.ascontiguousarray(ul.transpose(1, 0, 2).reshape(128, 512))

        T8 = np.zeros((8, TC), dtype=F32)
        for hc in range(9):          # levels 1..9, children = level-hc nodes
            w = LPB >> hc
            base = OFFS[hc] + c * (LPC >> hc)
            seg = bl[base: base + (LPC >> hc)].reshape(8, w)
            T8[:, int(BLK_OFF[hc]): int(BLK_OFF[hc]) + w] = seg[:, _bitrev(w)]
        T8[:, C_L10] = bl[OFFS[9] + c * 8: OFFS[9] + c * 8 + 8]
        T8[0:4, C_L11] = bl[OFFS[10] + c * 4: OFFS[10] + c * 4 + 4]
        T8[0:2, C_L12] = bl[OFFS[11] + c * 2: OFFS[11] + c * 2 + 2]
        T8[0, C_OWN] = bl[OFFS[12] + c]

        # per-core level-13 child weights: left (even core) vs right (odd)
        if c % 2 == 0:
            wt = np.concatenate([Wr.T, -W2.T], axis=1)
        else:
            wt = np.concatenate([gW.T, -gW2.T], axis=1)

        t8e = np.concatenate([T8, np.zeros((8, 128), F32)], axis=1)
        t8e = bf(t8e)
        t8e[:, TC:TC + 128] = expd
        in_maps.append({"t8e": t8e, "u_leaf": u_leaf, "wtail": bf(wt),
                        "packA1": packA1, "packA2": packA2,
                        "packB": packB, "packF": packF})
    return in_maps


def build_nc(split_waits=True):
    f32 = mybir.dt.float32
    bf16 = mybir.dt.bfloat16
    AF = mybir.ActivationFunctionType
    OP = mybir.AluOpType
    nc = bass.Bass()

    t8e = nc.dram_tensor("t8e", [8, TC + 128], bf16, kind="ExternalInput")
    u_leaf = nc.dram_tensor("u_leaf", [128, 512], bf16, kind="ExternalInput")
    wtail = nc.dram_tensor("wtail", [16, 32], bf16, kind="ExternalInput")
    packA1 = nc.dram_tensor("packA1", [128, 512], bf16, kind="ExternalInput")
    packA2 = nc.dram_tensor("packA2", [128, 128], bf16, kind="ExternalInput")
    packB = nc.dram_tensor("packB", [128, 648], bf16, kind="ExternalInput")
    packF = nc.dram_tensor("packF", [128, 24], f32, kind="ExternalInput")
    out = nc.dram_tensor("out", [17, 1], f32, kind="ExternalOutput")

    with tile.TileContext(nc) as tc:
        with (
            tc.tile_pool(name="const", bufs=1) as cp,
            tc.tile_pool(name="sb", bufs=2) as sb,
            tc.tile_pool(name="big", bufs=1) as bigp,
            tc.tile_pool(name="psT", bufs=1, space="PSUM") as psT,
            tc.tile_pool(name="psU", bufs=1, space="PSUM") as psU,
            tc.tile_pool(name="psL", bufs=1, space="PSUM") as psL,
            tc.tile_pool(name="psR", bufs=1, space="PSUM") as psR,
            tc.tile_pool(name="psS", bufs=1, space="PSUM") as psS,
        ):
            # --- table preload: tiny Exp fires the exp-table DMA at t=0
            d0 = cp.tile([1, 1], f32, tag="d0")
            nc.vector.memset(d0[:], 0.0)
            dm = cp.tile([1, 1], f32, tag="dm")
            nc.scalar.activation(dm[:], d0[:], AF.Exp, bias=d0[0:1, 0:1])

            # --- early inputs (tiny): branch rows, broadcast matrix, biases
            sTE = cp.tile([8, TC + 128], bf16, tag="t8e")
            nc.sync.dma_start(sTE[:], t8e[:, :])
            sT8 = sTE[:, 0:TC]
            cE = sTE[:, TC:TC + 128]
            cF = cp.tile([128, 24], f32, tag="packF")
            nc.sync.dma_start(cF[:], packF[:, :])

            # --- broadcast t to all 128 partitions (one matmul per chunk,
            # bank-aligned PSUM tiles); chunk order = consumption order
            # (level-1 right half first)
            T0 = psT.tile([128, 512], f32, tag="T0")
            T1 = psT.tile([128, 512], f32, tag="T1")
            T2 = psT.tile([128, TC - 1024], f32, tag="T2")
            TCH = ((256, 256, T0, 256), (0, 256, T0, 0),
                   (512, 512, T1, 0), (1024, TC - 1024, T2, 0))
            for lo, w, tt, to in TCH:
                nc.tensor.matmul(tt[:, to:to + w], cE, sT8[:, lo:lo + w],
                                 start=True, stop=True)

            # --- edge factors: exp all (table resident via the dummy),
            # then sin all (one load), scales/biases fused into ACT
            sE = bigp.tile([128, TC], f32, tag="sE")
            sC = bigp.tile([128, TC], f32, tag="sC")
            sSn = bigp.tile([128, TC], f32, tag="sSn")
            for lo, w, tt, to in TCH:
                nc.scalar.activation(sE[:, lo:lo + w], tt[:, to:to + w],
                                     AF.Exp, bias=cF[:, F_BOOST:F_BOOST + 1],
                                     scale=cF[:, F_A:F_A + 1])
            for lo, w, tt, to in TCH:
                nc.scalar.activation(sC[:, lo:lo + w], tt[:, to:to + w],
                                     AF.Sin, bias=cF[:, F_PI2:F_PI2 + 1],
                                     scale=cF[:, F_B:F_B + 1])
                nc.scalar.activation(sSn[:, lo:lo + w], tt[:, to:to + w],
                                     AF.Sin, bias=cF[:, F_ZERO:F_ZERO + 1],
                                     scale=cF[:, F_B:F_B + 1])

            # --- big inputs stream in behind the tiny ones
            cU = cp.tile([128, 512], bf16, tag="u_leaf")
            nc.sync.dma_start(cU[:], u_leaf[:, :])
            cA1 = cp.tile([128, 512], bf16, tag="packA1")
            nc.sync.dma_start(cA1[:], packA1[:, :])
            cA2 = cp.tile([128, 128], bf16, tag="packA2")
            nc.sync.dma_start(cA2[:], packA2[:, :])
            cB = cp.tile([128, 648], bf16, tag="packB")
            nc.sync.dma_start(cB[:], packB[:, :])
            cWt = cp.tile([16, 32], bf16, tag="wtail")
            nc.sync.dma_start(cWt[:], wtail[:, :])
            pobs1 = psL.tile([1, 1], f32, tag="YL")
            nc.tensor.matmul(pobs1[:], cU[0:1, 0:1], cU[0:1, 0:1],
                             start=True, stop=False)
            nc.tensor.matmul(pobs1[:], cA1[0:1, 0:1], cA1[0:1, 0:1],
                             start=False, stop=False)
            nc.tensor.matmul(pobs1[:], cA2[0:1, 0:1], cA2[0:1, 0:1],
                             start=False, stop=True)
            pobs2 = psR.tile([1, 1], f32, tag="YR")
            nc.tensor.matmul(pobs2[:], cB[0:1, 0:1], cB[0:1, 0:1],
                             start=True, stop=False)
            nc.tensor.matmul(pobs2[:], cF[0:1, 0:1], cF[0:1, 0:1],
                             start=False, stop=True)

            EC = bigp.tile([128, TC], f32, tag="EC")
            ES = bigp.tile([128, TC], f32, tag="ES")

            def eces(lo, w):
                nc.vector.tensor_mul(EC[:, lo:lo + w], sE[:, lo:lo + w],
                                     sC[:, lo:lo + w])
                nc.vector.tensor_mul(ES[:, lo:lo + w], sE[:, lo:lo + w],
                                     sSn[:, lo:lo + w])

            wl = cA1[:, A_WL:A_WL + 128]
            w2l = cA1[:, A_W2L:A_W2L + 128]
            wr = cA1[:, A_WR:A_WR + 128]
            w2r = cA1[:, A_W2R:A_W2R + 128]

            def level_cols(V, lo, wp, uL=None, uR=None, out_dt=bf16):
                """One sweep level, 128 partitions, contiguous halves.
                The u-matmul runs right-half first so the m-chain starts
                after a half-width matmul; the YR bounce is a DVE copy
                (cheaper than ACT and one fewer cross-engine hop)."""
                if V is not None:
                    pU = psU.tile([128, 2 * wp], f32, tag="U")
                    nc.tensor.matmul(pU[:, wp:2 * wp], cA2[:],
                                     V[:, wp:2 * wp], start=True, stop=True)
                    nc.tensor.matmul(pU[:, 0:wp], cA2[:], V[:, 0:wp],
                                     start=True, stop=True)
                    uL, uR = pU[:, 0:wp], pU[:, wp:2 * wp]
                m1R = sb.tile([128, wp], bf16, tag="m1R")
                nc.vector.tensor_mul(m1R[:], EC[:, lo + wp:lo + 2 * wp], uR)
                m2R = sb.tile([128, wp], bf16, tag="m2R")
                nc.vector.tensor_mul(m2R[:], ES[:, lo + wp:lo + 2 * wp], uR)
                m1L = sb.tile([128, wp], bf16, tag="m1L")
                nc.vector.tensor_mul(m1L[:], EC[:, lo:lo + wp], uL)
                m2L = sb.tile([128, wp], bf16, tag="m2L")
                nc.vector.tensor_mul(m2L[:], ES[:, lo:lo + wp], uL)
                pYR = psR.tile([128, wp], f32, tag="YR")
                nc.tensor.matmul(pYR[:], wr, m1R[:], start=True, stop=False)
                nc.tensor.matmul(pYR[:], w2r, m2R[:], start=False, stop=True)
                sYR = sb.tile([128, wp], bf16, tag="sYR")
                nc.scalar.activation(sYR[:], pYR[:], AF.Copy)
                pYL = psL.tile([128, wp], f32, tag="YL")
                nc.tensor.matmul(pYL[:], wl, m1L[:], start=True, stop=False)
                nc.tensor.matmul(pYL[:], w2l, m2L[:], start=False, stop=True)
                Vn = sb.tile([128, wp], out_dt, tag="V")
                nc.vector.tensor_mul(Vn[:], pYL[:], sYR[:])
                return Vn

            # --- level 1: leaf u-vectors come from the host; EC/ES chunks
            # interleave with the m-multiplies in consumption order, all
            # on DVE (concurrent GPSIMD traffic slows both ~2.5x)
            # level-1 factor fusion: m = (sE . u) . sin -- the exp-side
            # product runs ~2us before the sins land, taking one DVE op
            # off the critical chain (the EC/ES tiles' first 512 cols are
            # never materialized)
            euR = sb.tile([128, 256], f32, tag="euR")
            nc.vector.tensor_mul(euR[:], sE[:, 256:512], cU[:, 256:512])
            euL = sb.tile([128, 256], f32, tag="euL")
            nc.vector.tensor_mul(euL[:], sE[:, 0:256], cU[:, 0:256])
            m1R = sb.tile([128, 256], bf16, tag="m1R")
            nc.vector.tensor_mul(m1R[:], euR[:], sC[:, 256:512])
            m2R = sb.tile([128, 256], bf16, tag="m2R")
            nc.vector.tensor_mul(m2R[:], euR[:], sSn[:, 256:512])
            m1L = sb.tile([128, 256], bf16, tag="m1L")
            nc.vector.tensor_mul(m1L[:], euL[:], sC[:, 0:256])
            m2L = sb.tile([128, 256], bf16, tag="m2L")
            nc.vector.tensor_mul(m2L[:], euL[:], sSn[:, 0:256])
            pYR = psR.tile([128, 256], f32, tag="YR")
            nc.tensor.matmul(pYR[:], wr, m1R[:], start=True, stop=False)
            nc.tensor.matmul(pYR[:], w2r, m2R[:], start=False, stop=True)
            sYR = sb.tile([128, 256], bf16, tag="sYR")
            nc.scalar.activation(sYR[:], pYR[:], AF.Copy)
            pYL = psL.tile([128, 256], f32, tag="YL")
            nc.tensor.matmul(pYL[:], wl, m1L[:], start=True, stop=False)
            nc.tensor.matmul(pYL[:], w2l, m2L[:], start=False, stop=True)
            V = sb.tile([128, 256], bf16, tag="V")
            nc.vector.tensor_mul(V[:], pYL[:], sYR[:])

            # --- levels 2..9; remaining EC/ES chunks slot in just ahead
            # of the level that first needs them
            lsW = None
            for h in range(2, 10):
                wc = BLK_W[h - 1]
                wp = wc // 2
                lo = int(BLK_OFF[h - 1])
                if h == 2:
                    eces(512, 256)
                elif h == 3:
                    eces(768, TC - 768)
                if h == 8:
                    praw = level_cols(V, lo, wp)
                    pSb = psU.tile([128, wp], f32, tag="U")
                    nc.tensor.matmul(pSb[:], cB[:, B_ONB:B_ONB + 128],
                                     praw[:], start=True, stop=True)
                    pSc = psS.tile([8, wp], f32, tag="S")
                    nc.tensor.matmul(pSc[:], cB[:, B_ONC:B_ONC + 8],
                                     praw[:], start=True, stop=True)
                    rb = sb.tile([128, wp], f32, tag="rb")
                    nc.vector.reciprocal(rb[:], pSb[:])
                    V = sb.tile([128, wp], bf16, tag="V")
                    nc.vector.tensor_mul(V[:], praw[:], rb[:])
                    lsW = sb.tile([8, wp], f32, tag="lsW")
                    nc.scalar.activation(lsW[:], pSc[:], AF.Ln)
                else:
                    V = level_cols(V, lo, wp)

            # logscale: fold h=8's two parents, then sum the 8 blocks
            ls9 = sb.tile([8, 1], f32, tag="ls9")
            nc.gpsimd.tensor_add(ls9[:], lsW[:, 0:1], lsW[:, 1:2])
            ptot = psS.tile([1, 1], f32, tag="S")
            nc.tensor.matmul(ptot[:], cF[0:8, F_ONE8:F_ONE8 + 1], ls9[:],
                             start=True, stop=True)
            stot = sb.tile([1, 1], f32, tag="stot")
            nc.vector.tensor_copy(stot[:], ptot[:])
            nc.sync.dma_start(out[16:17, 0:1], stot[:])

            # --- levels 10..12 (block domain: pairs of partition blocks)
            col = C_L10
            for kp, op in ((128, 64), (64, 32), (32, 16)):
                pU = psU.tile([kp, 1], f32, tag="U")
                nc.tensor.matmul(pU[:], cA2[0:kp, 0:kp], V[:],
                                 start=True, stop=True)
                m1 = sb.tile([kp, 1], bf16, tag="m1R")
                nc.vector.tensor_mul(m1[:], EC[0:kp, col:col + 1], pU[:])
                m2 = sb.tile([kp, 1], bf16, tag="m2R")
                nc.vector.tensor_mul(m2[:], ES[0:kp, col:col + 1], pU[:])
                pYR = psR.tile([op, 1], f32, tag="YR")
                nc.tensor.matmul(pYR[:], cB[0:kp, B_R1:B_R1 + op], m1[:],
                                 start=True, stop=False)
                nc.tensor.matmul(pYR[:], cB[0:kp, B_R2:B_R2 + op], m2[:],
                                 start=False, stop=True)
                sYR = sb.tile([op, 1], bf16, tag="sYR")
                nc.vector.tensor_copy(sYR[:], pYR[:])
                pYL = psL.tile([op, 1], f32, tag="YL")
                nc.tensor.matmul(pYL[:], cB[0:kp, B_L1:B_L1 + op], m1[:],
                                 start=True, stop=False)
                nc.tensor.matmul(pYL[:], cB[0:kp, B_L2:B_L2 + op], m2[:],
                                 start=False, stop=True)
                V = sb.tile([op, 1], bf16, tag="V")
                nc.vector.tensor_mul(V[:], pYL[:], sYR[:])
                col += 1

            # --- own level-13 child transform (this core's root edge;
            # W vs gW by core parity baked into the wtail input)
            pU = psU.tile([16, 1], f32, tag="U")
            nc.tensor.matmul(pU[:], cA2[0:16, 0:16], V[:],
                             start=True, stop=True)
            m1 = sb.tile([16, 1], bf16, tag="m1R")
            nc.vector.tensor_mul(m1[:], EC[0:16, C_OWN:C_OWN + 1], pU[:])
            m2 = sb.tile([16, 1], bf16, tag="m2R")
            nc.vector.tensor_mul(m2[:], ES[0:16, C_OWN:C_OWN + 1], pU[:])
            pYo = psL.tile([16, 1], f32, tag="YL")
            nc.tensor.matmul(pYo[:], cWt[:, 0:16], m1[:],
                             start=True, stop=False)
            nc.tensor.matmul(pYo[:], cWt[:, 16:32], m2[:],
                             start=False, stop=True)
            yown = sb.tile([16, 1], f32, tag="yown")
            nc.vector.tensor_copy(yown[:], pYo[:])
            nc.sync.dma_start(out[0:16, 0:1], yown[:])

    if split_waits:
        _split_multi_waits(nc)
    return nc


def _host_top_combine(results, branch_lens, Q, growth_rates):
    """Final 7-node combine (levels 14,15 + root) of the per-core
    contributions, in f64 -- O(S^2) unsharding glue."""
    bl = np.asarray(branch_lens, dtype=np.float64)
    Q64 = np.asarray(Q, dtype=np.float64)
    g64 = np.asarray(growth_rates, dtype=np.float64)
    R = Q64 - np.diag(g64)
    Wr, Winv, a, bsig, swap = _real_eig(R)
    P = np.zeros((S, S))
    P[np.arange(S), swap] = 1.0
    W2 = Wr @ P
    b2 = bsig[swap]

    def edge(t, v, g=None):
        u = Winv @ v
        y = Wr @ (np.exp(a * t) * np.cos(bsig * t) * u) \
            + W2 @ (np.exp(a * t) * np.sin(b2 * t) * u)
        return y if g is None else g * y

    ys = [np.asarray(r["out"], dtype=np.float64)[0:16, 0] for r in results]
    tot = sum(float(np.asarray(r["out"], dtype=np.float64)[16, 0])
              for r in results)
    corr = np.float64(NCORES * DEV_EDGES) * np.float64(np.float32(BOOST))
    v14 = [ys[2 * j] * ys[2 * j + 1] for j in range(4)]
    y14 = [edge(bl[OFFS[13] + k], v14[k], g64 if k % 2 else None)
           for k in range(4)]
    v15 = [y14[0] * y14[1], y14[2] * y14[3]]
    y15 = [edge(bl[OFFS[14] + k], v15[k], g64 if k % 2 else None)
           for k in range(2)]
    v16 = y15[0] * y15[1]
    yroot = edge(bl[OFFS[15]], v16)
    return (np.log(yroot) + (tot - corr)).astype(F32)


def kernel(postorder, children, parents, branch_lens, init_partials, Q,
           levels, growth_rates, *, _trace=False):
    in_maps = _host_prep(branch_lens, init_partials, Q, growth_rates)
    nc = build_nc()
    res = run_bass_kernel_spmd(nc, in_maps, core_ids=list(range(NCORES)),
                               trace=_trace)
    out = _host_top_combine(res.results, branch_lens, Q, growth_rates)
    if _trace:
        kernel.last_exec_time_ns = res.exec_time_ns
        kernel.last_results = res
    return out
